# revision 7
# baseline (speedup 1.0000x reference)
"""Trainium2 Bass kernel for nn_Attention_28750511080014 (GQA attention), v2.

Reference semantics (replicated exactly, including the noncanonical plain
reshape):
  qkv = x @ w_attn.T; q/k/v split with plain reshape (no transpose);
  causal softmax attention with repeat_interleave(4) kv heads;
  y -> transpose -> [B,S,1024] @ w_proj.T

Sharding: 8 cores = 2 batches x 4 kv-groups; core (b,g) computes 4 q-heads +
1 kv-head from x rows [512g, 512g+512) and a partial output projection over
its 256 y2d columns; host sums 4 partials per batch.

v2 over the f32r baseline (146.9us):
  - q/k QKV columns via fp8 DoubleRow matmuls (x8/w8 host-quantized; wq
    scaled x64, wk x32 to dodge fp8 subnormals; descale folded into the
    softmax exp scale 1/2048). v columns stay f32r (fp8 V fails the gate).
  - Scores via fp8 DoubleRow with a zero second plane (contraction 64):
    0.5 cycles/row.
  - exp split between the Scalar engine (exact table exp) and DVE
    (Schraudolph bit trick: one tensor_scalar mult+add to int16, bitcast as
    bf16), breaking the single-engine exp bottleneck.
  - Causal trimming at 128-column granularity on diagonal tiles for scores,
    exp, masks and PV; masks shrink to one [128,2,128] triangle multiply.
  - PV with stationary V1 (64 dims + ones column -> free denominator row),
    moving P; normalization: DVE reciprocal + gpsimd partition_broadcast +
    DVE multiply into y2dT (f32r); f32r output projection, deferred into the
    next quarter's PE slack; PSUM->SBUF out copies alternate Scalar/DVE.
"""

import sys
import numpy as np
from contextlib import ExitStack

for _p in ("/opt/trn_rl_repo",):
    if _p not in sys.path:
        sys.path.insert(0, _p)

B, S, H = 2, 2048, 1024
NQ, NKV, HD = 16, 4, 64
GHD = 256
G = 4
SB = 512
NCORES = 8
NH = 4              # local q heads per core
KT = 16             # k-tiles of 128 over S
SCALE = 0.125
QS = 64.0           # fp8 scale on wq (net 8x with SCALE folded)
KS = 32.0           # fp8 scale on wk
ESC = 1.0 / (QS * SCALE * KS * 8.0)   # 1/2048 exp descale

# Schraudolph bf16-space exp bit construction (input is 2048x true score)
SCH_A = float(2 ** 7 / np.log(2)) * ESC
SCH_B = float((127 - 0.04367) * 2 ** 7)

# exp tile i goes to DVE when i % MOD in RES
import os
EXP_DVE_MOD = int(os.environ.get("K2_EXP_MOD", "4"))
EXP_DVE_RES = tuple(
    int(v) for v in os.environ.get("K2_EXP_RES", "3").split(",") if v != "")
K2_QCOPY_ACT = int(os.environ.get("K2_QCOPY_ACT", "1"))   # half on ACT
K2_EXP_DIAG = tuple(
    int(v) for v in os.environ.get("K2_EXP_DIAG", "3").split(",") if v != "")
K2_SC_PRIO = int(os.environ.get("K2_SC_PRIO", "0"))
K2_OUT_ACT = int(os.environ.get("K2_OUT_ACT", "0"))       # alternate ACT/DVE
K2_DRAIN = int(os.environ.get("K2_DRAIN", "1"))
K2_MASK_POOL = int(os.environ.get("K2_MASK_POOL", "0"))
K2_DMA_BCAST = int(os.environ.get("K2_DMA_BCAST", "0"))
K2_EXP_BOTH = int(os.environ.get("K2_EXP_BOTH", "0"))
K2_EXP_KT = int(os.environ.get("K2_EXP_KT", "0"))
K2_QCOPY_ALL_ACT = int(os.environ.get("K2_QCOPY_ALL_ACT", "0"))
K2_DVE_PRIO = int(os.environ.get("K2_DVE_PRIO", "0"))
K2_H_DESC = int(os.environ.get("K2_H_DESC", "0"))
K2_V_DEFER = int(os.environ.get("K2_V_DEFER", "0"))
K2_PV_LAG = int(os.environ.get("K2_PV_LAG", "2"))
K2_MASK_LAG = int(os.environ.get("K2_MASK_LAG", "0"))
K2_PTQ = int(os.environ.get("K2_PTQ", "18"))
K2_QCOPY_SPLIT = int(os.environ.get("K2_QCOPY_SPLIT", "0"))
K2_PROJ_SPLIT = int(os.environ.get("K2_PROJ_SPLIT", "1"))
K2_NORM_SPLIT = int(os.environ.get("K2_NORM_SPLIT", "1"))
K2_EXP_SPLIT = tuple(
    int(v) for v in os.environ.get("K2_EXP_SPLIT", "").split(",") if v != "")

_NC = None


def _build_body(ctx, tc, dram):
    import concourse.bass as bass
    import concourse.mybir as mybir

    nc = tc.nc
    dt = mybir.dt
    f32 = dt.float32
    f32r = dt.float32r
    bf16 = dt.bfloat16
    fp8 = dt.float8e4
    i16 = dt.int16
    DRm = mybir.MatmulPerfMode.DoubleRow
    Exp = mybir.ActivationFunctionType.Exp

    # ---- pools ----
    inp = ctx.enter_context(tc.tile_pool(name="inputs", bufs=1))
    cpool = ctx.enter_context(tc.tile_pool(name="consts", bufs=1))
    psA = ctx.enter_context(tc.tile_pool(name="psA", bufs=2, space="PSUM"))
    psB = ctx.enter_context(tc.tile_pool(name="psB", bufs=4, space="PSUM"))
    ptp = ctx.enter_context(tc.tile_pool(name="ptp", bufs=8))
    ptq = ctx.enter_context(tc.tile_pool(name="ptq", bufs=K2_PTQ))

    # ---- SBUF input tensors ----
    x8_sb = inp.tile([128, 4, 2, SB], fp8, tag="x8")
    w8_sb = inp.tile([128, 4, 2, 1280], fp8, tag="w8")
    xt_sb = inp.tile([128, 8, SB], f32r, tag="xt")
    wv_sb = inp.tile([128, 8, GHD], f32r, tag="wv")
    wp_sb = inp.tile([128, 2, H], f32r, tag="wp")
    tri_sb = inp.tile([128, 256], bf16, tag="tri")
    idn64_sb = inp.tile([64, 64], f32r, tag="idn64")

    nc.sync.dma_start(x8_sb[...], dram["x8"][...])
    nc.sync.dma_start(w8_sb[:, :, :, 1024:1280],
                      dram["w8"][:, :, :, 1024:1280])
    nc.sync.dma_start(w8_sb[:, :, :, 0:1024], dram["w8"][:, :, :, 0:1024])
    nc.sync.dma_start(tri_sb[...], dram["tri"][...])
    nc.sync.dma_start(idn64_sb[...], dram["idn64"][...])
    nc.sync.dma_start(
        xt_sb[:, :, :],
        dram["xt"][:, :].rearrange("(ht p) s -> p ht s", p=128))
    nc.sync.dma_start(wv_sb[...], dram["wv"][...])
    nc.sync.dma_start(
        wp_sb[:, :, :],
        dram["wp"][:, :].rearrange("(ct p) o -> p ct o", p=128))
    outT = dram["outt"]
    rrs = dram["rrs"]

    # ---- persistent SBUF ----
    q8 = cpool.tile([64, 2, NH, S], fp8, tag="q8")
    k8 = cpool.tile([64, 2, S], fp8, tag="k8")
    vTs = cpool.tile([64, S], f32r, tag="vT")
    v1 = cpool.tile([128, KT, HD + 1], bf16, tag="v1")
    y2dT = cpool.tile([128, 2, S], f32r, tag="y2dT")

    nc.gpsimd.memset(q8[:, 1, :, :], 0.0)
    nc.gpsimd.memset(k8[:, 1, :], 0.0)
    nc.vector.memset(v1[:, :, HD:HD + 1], 1.0)
    warm = cpool.tile([1, 8], f32, tag="warm")
    nc.vector.memset(warm[:, :], 0.0)
    nc.scalar.activation(warm[0:1, 0:8], warm[0:1, 0:8], Exp)

    tri3 = tri_sb[:, :].rearrange("p (b z) -> p b z", z=128)

    # ---- QKV projection ----
    def kq_oc(oc, is_k):
        ps = psB.tile([128, SB], f32, tag="sc")
        c0 = 1024 + oc * 128 if is_k else oc * 128
        for dr in range(4):
            nc.tensor.matmul(
                ps[:, :],
                w8_sb[:, dr, :, c0:c0 + 128],
                x8_sb[:, dr, :, :],
                start=(dr == 0), stop=(dr == 3),
                perf_mode=DRm,
            )
        for half in range(2):
            src = ps[64 * half:64 * half + 64, :]
            c = 2 * oc + half
            if is_k:
                nc.vector.tensor_copy(out=k8[0:64, 0, c:S:4], in_=src)
            else:
                s3 = src.rearrange("d (j r) -> d j r", r=128)
                if K2_QCOPY_SPLIT:
                    for jp in range(2):
                        dapj = q8[0:64, 0, 2 * jp:2 * jp + 2, c:S:16]
                        s3j = s3[:, 2 * jp:2 * jp + 2, :]
                        if (half + jp) % 2 == 0 and K2_QCOPY_ACT:
                            nc.scalar.copy(dapj, s3j)
                        else:
                            nc.vector.tensor_copy(out=dapj, in_=s3j)
                else:
                    dap = q8[0:64, 0, :, c:S:16]
                    if K2_QCOPY_ALL_ACT:
                        nc.scalar.copy(dap, s3)
                    elif half == 0 or not K2_QCOPY_ACT:
                        nc.vector.tensor_copy(out=dap, in_=s3)
                    else:
                        nc.scalar.copy(dap, s3)

    def v_oc(oc):
        ps = psB.tile([128, SB], f32, tag="sc")
        for ht in range(8):
            nc.tensor.matmul(
                ps[:, :],
                wv_sb[:, ht, oc * 128:(oc + 1) * 128],
                xt_sb[:, ht, :],
                start=(ht == 0), stop=(ht == 7),
            )
        for half in range(2):
            c = 2 * oc + half
            nc.vector.tensor_copy(out=vTs[0:64, c:S:4],
                                  in_=ps[64 * half:64 * half + 64, :])

    for oc in range(2):
        kq_oc(oc, is_k=True)
    for oc in range(8):
        kq_oc(oc, is_k=False)
    ctx_v = tc.high_priority(offset=K2_V_DEFER) if K2_V_DEFER else None
    if ctx_v is not None:
        ctx_v.__enter__()
    for oc in range(2):
        v_oc(oc)

    # v transposes: vTs [64, S] -> v1 [128, kt, 64] bf16, packed 8 per bank
    for grp in range(2):
        tpv = psB.tile([128, SB], f32r, tag="sc")
        for sl in range(8):
            kt = grp * 8 + sl
            with nc.allow_low_precision(reason="transpose is data movement"):
                nc.tensor.transpose(
                    tpv[:, sl * 64:sl * 64 + 64],
                    vTs[0:64, 128 * kt:128 * (kt + 1)],
                    idn64_sb[:, :],
                )
        nc.vector.tensor_copy(
            out=v1[:, grp * 8:grp * 8 + 8, 0:HD],
            in_=tpv[:, :].rearrange("p (sl d) -> p sl d", d=64),
        )
    if ctx_v is not None:
        ctx_v.__exit__(None, None, None)

    # ---- attention ----
    pending = []
    done_h = [None]
    exp_idx = [0]
    cp_idx = [0]
    bc_idx = [0]

    def drain(n):
        for _ in range(n):
            if pending:
                pending.pop(0)()

    def emit_norm_head(p, h, m, yt):
        def go():
                rr = ptp.tile([1, SB], f32, tag="rr")
                nc.vector.reciprocal(rr[0:1, :], yt[64:65, 0:SB])
                rbs = ptp.tile([64, SB], f32, tag="rbs")
                if K2_DMA_BCAST:
                    slot = bc_idx[0] % 16
                    bc_idx[0] += 1
                    nc.sync.dma_start(rrs[slot:slot + 1, :], rr[0:1, :])
                    nc.sync.dma_start(
                        rbs[0:64, :],
                        rrs[slot:slot + 1, :].broadcast_to([64, SB]))
                else:
                    nc.gpsimd.partition_broadcast(
                        rbs[0:64, :], rr[0:1, :], channels=64)
                nc.vector.tensor_mul(
                    y2dT[64 * m:64 * m + 64, p, 512 * h:512 * h + 512],
                    yt[0:64, 0:SB],
                    rbs[0:64, :],
                )
        return go

    def emit_norm(p, h, yts):
        def go():
            emit_norm_head(p, h, 0, yts[0])()
            emit_norm_head(p, h, 1, yts[1])()
        return go

    def proj_sub(hq, ot0, sub, obref):
        def go():
            ctx2 = tc.high_priority(offset=-120)
            ctx2.__enter__()
            if sub == 0:
                obref[0] = ptp.tile([128, 2, SB], f32, tag="ob", name="ob")
            ob = obref[0]
            ot = ot0 + sub
            pp = psB.tile([128, SB], f32, tag="sc")
            for ct in range(2):
                nc.tensor.matmul(
                    pp[:, :],
                    wp_sb[:, ct, ot * 128:(ot + 1) * 128],
                    y2dT[:, ct, 512 * hq:512 * hq + 512],
                    start=(ct == 0), stop=(ct == 1),
                )
            use_act = (K2_OUT_ACT or hq == 3) and cp_idx[0] % 2 == 0
            if use_act:
                nc.scalar.copy(ob[:, sub, :], pp[:, :])
            else:
                nc.vector.tensor_copy(out=ob[:, sub, :], in_=pp[:, :])
            cp_idx[0] += 1
            if sub == 1:
                nc.sync.dma_start(
                    outT[ot0 * 128:(ot0 + 2) * 128,
                         512 * hq:512 * hq + 512]
                    .rearrange("(sub p) q -> p sub q", p=128),
                    ob[:, :, :],
                )
            ctx2.__exit__(None, None, None)
        return go

    def proj_piece(hq, ot0):
        if not K2_PROJ_SPLIT:
            a, b = None, None

            def go_both():
                obref = [None]
                proj_sub(hq, ot0, 0, obref)()
                proj_sub(hq, ot0, 1, obref)()
            return go_both
        return None

    def emit_pv(pend):
        yts, kt, pt3, kt_max, t0 = pend
        for m in range(2):
            nc.tensor.matmul(
                yts[m][0:HD + 1, t0:SB],
                v1[:, kt, :],
                pt3[:, m, t0:],
                start=(kt == 0), stop=(kt == kt_max),
            )

    for h in (list(range(3, -1, -1)) if K2_H_DESC else list(range(4))):
        kt_max = 4 * h + 3
        for p in range(2):
            ytA = psB.tile([HD + 1, SB], f32, tag="sc")
            ytB = psB.tile([HD + 1, SB], f32, tag="sc")
            yts = (ytA, ytB)
            pends = []
            pend_masks = []
            for kt in range(kt_max + 1):
                mm = kt - 4 * h
                t0 = 128 * mm if mm >= 0 else 0
                st = psA.tile([128, 1024], f32, tag="st")
                st3 = st[:, :].rearrange("p (m z) -> p m z", z=512)
                ctx_p = tc.high_priority(offset=K2_SC_PRIO) if K2_SC_PRIO \
                    else None
                if ctx_p is not None:
                    ctx_p.__enter__()
                for m in range(2):
                    j = 2 * p + m
                    nc.tensor.matmul(
                        st[:, 512 * m + t0:512 * (m + 1)],
                        k8[:, :, 128 * kt:128 * (kt + 1)],
                        q8[:, :, j, 512 * h + t0:512 * h + 512],
                        start=True, stop=True,
                        perf_mode=DRm,
                    )
                if ctx_p is not None:
                    ctx_p.__exit__(None, None, None)
                pt = ptq.tile([128, 1024], bf16, tag="pt")
                pt3 = pt[:, :].rearrange("p (m z) -> p m z", z=512)
                i = exp_idx[0]
                exp_idx[0] += 1
                if K2_EXP_DIAG:
                    use_dve = mm in K2_EXP_DIAG or (
                        K2_EXP_BOTH and mm < 0
                        and i % EXP_DVE_MOD in EXP_DVE_RES) or (
                        K2_EXP_KT and mm < 0 and kt % K2_EXP_KT == 1)
                else:
                    use_dve = i % EXP_DVE_MOD in EXP_DVE_RES
                if mm < 0 and kt in K2_EXP_SPLIT:
                    nc.scalar.activation(pt3[:, 0:1, t0:], st3[:, 0:1, t0:],
                                         Exp, scale=ESC)
                    nc.vector.tensor_scalar(
                        out=pt3[:, 1:2, t0:].bitcast(i16),
                        in0=st3[:, 1:2, t0:],
                        scalar1=SCH_A, scalar2=SCH_B,
                        op0=mybir.AluOpType.mult, op1=mybir.AluOpType.add)
                elif use_dve:
                    ctx_d = tc.high_priority(offset=K2_DVE_PRIO) \
                        if K2_DVE_PRIO else None
                    if ctx_d is not None:
                        ctx_d.__enter__()
                    nc.vector.tensor_scalar(
                        out=pt3[:, :, t0:].bitcast(i16),
                        in0=st3[:, :, t0:],
                        scalar1=SCH_A, scalar2=SCH_B,
                        op0=mybir.AluOpType.mult, op1=mybir.AluOpType.add)
                    if ctx_d is not None:
                        ctx_d.__exit__(None, None, None)
                else:
                    nc.scalar.activation(pt3[:, :, t0:], st3[:, :, t0:],
                                         Exp, scale=ESC)
                def mk_mask(pt3=pt3, t0=t0):
                    def go():
                        if K2_MASK_POOL:
                            nc.gpsimd.tensor_mul(
                                pt3[:, :, t0:t0 + 128],
                                pt3[:, :, t0:t0 + 128],
                                tri3)
                        else:
                            nc.vector.tensor_mul(
                                pt3[:, :, t0:t0 + 128],
                                pt3[:, :, t0:t0 + 128],
                                tri3)
                    return go
                if mm >= 0:
                    if K2_MASK_LAG:
                        pend_masks.append(mk_mask())
                    else:
                        mk_mask()()
                drain(K2_DRAIN)
                if len(pend_masks) > K2_MASK_LAG:
                    pend_masks.pop(0)()
                if len(pends) >= K2_PV_LAG:
                    emit_pv(pends.pop(0))
                pends.append((yts, kt, pt3, kt_max, t0))
            for go in pend_masks:
                go()
            for pd in pends:
                emit_pv(pd)
            if K2_NORM_SPLIT:
                pending.append(emit_norm_head(p, h, 0, yts[0]))
                pending.append(emit_norm_head(p, h, 1, yts[1]))
            else:
                pending.append(emit_norm(p, h, yts))
            if p == 1:
                if done_h[0] is not None:
                    for ot0 in range(0, 8, 2):
                        if K2_PROJ_SPLIT:
                            obref = [None]
                            pending.append(proj_sub(done_h[0], ot0, 0, obref))
                            pending.append(proj_sub(done_h[0], ot0, 1, obref))
                        else:
                            pending.append(proj_piece(done_h[0], ot0))
                done_h[0] = h
    drain(len(pending))
    for ot0 in range(0, 8, 2):
        obref = [None]
        proj_sub(done_h[0], ot0, 0, obref)()
        proj_sub(done_h[0], ot0, 1, obref)()


def _build():
    import concourse.tile as tile
    from concourse import bacc
    import concourse.mybir as mybir

    dt = mybir.dt
    nc = bacc.Bacc("TRN2", target_bir_lowering=False, debug=False,
                   num_devices=NCORES)
    dram = {
        "x8": nc.dram_tensor("x8", [128, 4, 2, SB], dt.float8e4,
                             kind="ExternalInput").ap(),
        "w8": nc.dram_tensor("w8", [128, 4, 2, 1280], dt.float8e4,
                             kind="ExternalInput").ap(),
        "xt": nc.dram_tensor("xt", [H, SB], dt.float32r,
                             kind="ExternalInput").ap(),
        "wv": nc.dram_tensor("wv", [128, 8, GHD], dt.float32r,
                             kind="ExternalInput").ap(),
        "wp": nc.dram_tensor("wp", [GHD, H], dt.float32r,
                             kind="ExternalInput").ap(),
        "tri": nc.dram_tensor("tri", [128, 256], dt.bfloat16,
                              kind="ExternalInput").ap(),
        "idn64": nc.dram_tensor("idn64", [64, 64], dt.float32r,
                                kind="ExternalInput").ap(),
        "outt": nc.dram_tensor("outt", [H, S], dt.float32,
                               kind="ExternalOutput").ap(),
        "rrs": nc.dram_tensor("rrs", [16, SB], dt.float32,
                              kind="Internal").ap(),
    }

    with tile.TileContext(nc) as tc, ExitStack() as ctx:
        ctx.enter_context(
            nc.allow_low_precision(reason="fp8/bf16 rounding is intentional"))
        _build_body(ctx, tc, dram)
    nc.compile()
    return nc


def _get_nc():
    global _NC
    if _NC is None:
        _NC = _build()
    return _NC


def _host_inputs(x, w_attn, w_proj):
    import ml_dtypes
    FP8 = ml_dtypes.float8_e4m3
    BF16 = ml_dtypes.bfloat16
    x = np.asarray(x, np.float32)
    w_attn = np.asarray(w_attn, np.float32)
    w_proj = np.asarray(w_proj, np.float32)

    wq = w_attn[:H] * SCALE
    wT = np.concatenate([wq, w_attn[H:]], axis=0).T      # [1024, 1536]
    wqk = wT[:, :1280].copy()
    wqk[:, :1024] *= QS
    wqk[:, 1024:1280] *= KS
    w8 = np.ascontiguousarray(
        wqk.reshape(4, 2, 128, 1280).transpose(2, 0, 1, 3)).astype(FP8)
    wv = np.ascontiguousarray(
        wT[:, 1280:].reshape(8, 128, GHD).transpose(1, 0, 2))

    k_i = np.arange(128)[:, None]
    q_i = np.arange(128)[None, :]
    tri1 = (k_i <= q_i).astype(np.float32)
    tri = np.concatenate([tri1, tri1], axis=1).astype(BF16)  # [128, 256]
    idn64 = np.eye(64, dtype=np.float32)

    in_maps = []
    for c in range(NCORES):
        b, g = c // 4, c % 4
        xs = x[b, g * SB:(g + 1) * SB, :]                # [512, 1024]
        xT = np.ascontiguousarray(xs.T)                  # [1024, 512]
        x8 = np.ascontiguousarray(
            xT.reshape(4, 2, 128, SB).transpose(2, 0, 1, 3)).astype(FP8)
        wpT = np.ascontiguousarray(w_proj[:, g * GHD:(g + 1) * GHD].T)
        in_maps.append({"x8": x8, "w8": w8, "xt": xT, "wv": wv, "wp": wpT,
                        "tri": tri, "idn64": idn64})
    return in_maps


def _gather(results):
    out = np.zeros((B, S, H), np.float32)
    for c in range(NCORES):
        b = c // 4
        out[b] += results[c]["outt"].T
    return out


def kernel(x, w_attn, w_proj):
    from concourse.bass_utils import run_bass_kernel_spmd
    nc = _get_nc()
    in_maps = _host_inputs(x, w_attn, w_proj)
    res = run_bass_kernel_spmd(nc, in_maps, core_ids=list(range(NCORES)))
    return _gather(res.results)
